# revision 8
# baseline (speedup 1.0000x reference)
"""Cross-attention block (q/k/v proj -> gated softmax attention -> out proj
-> residual + LayerNorm) on 8 Trainium2 NeuronCores.

Sharding: data-parallel over batch (B=4) x query-row halves (2) = 8 shards.
Each core handles one (b, m-half): computes full k/v projections for its
batch, attention for all 16 heads over its 512 query rows, output
projection, residual + LayerNorm. No collectives; host concatenates.

Layout strategy (all matmul operands live in SBUF as float32r):
  - Activations are pre-transposed on host to contraction-major layouts:
      Qt  = Q[b].T          [D, M_SH]   (lhs rhs for q-proj)
      KVt = KV[b].T         [D, N]
      W*t = W.T             [D_in, D_out]
  - q/k projections produce head-major (o-major) tiles directly:
      qT [o, m], kT [o, n]  -> exactly the lhsT/rhs layouts attention needs.
  - Scores are computed TRANSPOSED: S^T[n, m] = kT_slice.T @ qT_slice, so
    softmax's additive gate bias (per n) is a per-partition ACT bias and
    exp(S/8 + gate) is a single fused ACT op from PSUM. No row-max pass
    (scores are O(+-8); exp is fp32-safe).
  - v is produced in natural [n, dh] layout with a ones-column appended per
    head, so P^T-matmul accumulates both attn@v AND the softmax denominator
    in one pass: psum[65, m] per head. Normalisation is folded in after PV.
  - PV output [dh, m] is k-major: stacked heads form the o-proj lhsT with no
    transposes anywhere in the kernel.
"""
import numpy as np

import concourse.bass as bass
import concourse.mybir as mybir
import concourse.tile as tile
from concourse import bacc
from concourse.bass_utils import run_bass_kernel_spmd

F32 = mybir.dt.float32
F32R = mybir.dt.float32r
AFT = mybir.ActivationFunctionType

B, M, N, D = 4, 1024, 2048, 1024
H, DH = 16, 64
M_SH = M // 2          # query rows per core
G = 4                  # head groups
HPG = H // G           # heads per group
OG = HPG * DH          # 256 output cols per group
KT = D // 128          # 8 contraction subtiles
LN_EPS = 1e-5
SCALE = 1.0 / np.sqrt(DH)
N_CORES = 8

_CACHE = {}


def build_nc():
    nc = bacc.Bacc("TRN2", target_bir_lowering=False, debug=False)

    qt = nc.dram_tensor("qt", [D, M_SH], F32, kind="ExternalInput")
    qn = nc.dram_tensor("qn", [M_SH, D], F32, kind="ExternalInput")
    kvt = nc.dram_tensor("kvt", [D, N], F32, kind="ExternalInput")
    wqt = nc.dram_tensor("wqt", [D, D], F32, kind="ExternalInput")
    wkt = nc.dram_tensor("wkt", [D, D], F32, kind="ExternalInput")
    wvt = nc.dram_tensor("wvt", [D, D], F32, kind="ExternalInput")
    wot = nc.dram_tensor("wot", [D, D], F32, kind="ExternalInput")
    gate = nc.dram_tensor("gate", [N], F32, kind="ExternalInput")
    bq = nc.dram_tensor("bq", [D], F32, kind="ExternalInput")
    bk = nc.dram_tensor("bk", [D], F32, kind="ExternalInput")
    bv = nc.dram_tensor("bv", [D], F32, kind="ExternalInput")
    bo = nc.dram_tensor("bo", [D], F32, kind="ExternalInput")
    gamma = nc.dram_tensor("gamma", [D], F32, kind="ExternalInput")
    beta = nc.dram_tensor("beta", [D], F32, kind="ExternalInput")
    onesc = nc.dram_tensor("onesc", [DH], F32, kind="ExternalInput")
    out = nc.dram_tensor("out", [M_SH, D], F32, kind="ExternalOutput")

    # contraction-major DRAM views [128, KT, x]
    qt_v = qt.rearrange("(s p) m -> p s m", p=128)
    kvt_v = kvt.rearrange("(s p) n -> p s n", p=128)
    wqt_v = wqt.rearrange("(s p) o -> p s o", p=128)
    wkt_v = wkt.rearrange("(s p) o -> p s o", p=128)
    wvt_v = wvt.rearrange("(s p) o -> p s o", p=128)
    wot_v = wot.rearrange("(s p) o -> p s o", p=128)

    with tile.TileContext(nc) as tc:
        with tc.tile_pool(name="consts", bufs=1) as consts, \
             tc.tile_pool(name="stream", bufs=2) as stream, \
             tc.tile_pool(name="wg", bufs=4) as wgp, \
             tc.tile_pool(name="qtg", bufs=2) as qtgp, \
             tc.tile_pool(name="ktg", bufs=2) as ktgp, \
             tc.tile_pool(name="vbig", bufs=1) as vbigp, \
             tc.tile_pool(name="pt", bufs=2) as ptp, \
             tc.tile_pool(name="aot", bufs=1) as aotp, \
             tc.tile_pool(name="small", bufs=2) as small, \
             tc.tile_pool(name="outst", bufs=2) as outst, \
             tc.tile_pool(name="ps", bufs=4, space="PSUM") as psp, \
             tc.tile_pool(name="pv", bufs=4, space="PSUM") as pvp:

            # ---- constants ----
            gate_sb = consts.tile([128, N // 128], F32, tag="gate")
            nc.sync.dma_start(gate_sb[:], gate.rearrange("(t p) -> p t", p=128))
            bq_sb = consts.tile([128, KT], F32, tag="bq")
            nc.sync.dma_start(bq_sb[:], bq.rearrange("(t p) -> p t", p=128))
            bk_sb = consts.tile([128, KT], F32, tag="bk")
            nc.sync.dma_start(bk_sb[:], bk.rearrange("(t p) -> p t", p=128))
            bv_b = consts.tile([128, D], F32, tag="bv")
            nc.sync.dma_start(bv_b[:], bv[None, :].to_broadcast((128, D)))
            bo_b = consts.tile([128, D], F32, tag="bo")
            nc.sync.dma_start(bo_b[:], bo[None, :].to_broadcast((128, D)))
            gamma_b = consts.tile([128, D], F32, tag="gamma")
            nc.sync.dma_start(gamma_b[:], gamma[None, :].to_broadcast((128, D)))
            beta_b = consts.tile([128, D], F32, tag="beta")
            nc.sync.dma_start(beta_b[:], beta[None, :].to_broadcast((128, D)))
            eps_sb = consts.tile([128, 1], F32, tag="eps")
            nc.vector.memset(eps_sb[:], LN_EPS)
            ones1 = consts.tile([1, DH], F32R, tag="ones1")
            nc.gpsimd.dma_start(ones1[:], onesc[None, :])

            # q activations, contraction-major, fp32r (loaded once)
            qt_sb = consts.tile([128, KT, M_SH], F32R, tag="qt")
            nc.gpsimd.dma_start(qt_sb[:], qt_v)

            # o-proj accumulator input: stacked head outputs [o, m]
            aot = aotp.tile([128, KT, M_SH], F32R, tag="aot")

            for g in range(G):
                ob = g * OG  # base output column of this group

                wq_g = wgp.tile([128, KT, OG], F32R, tag="w")
                nc.gpsimd.dma_start(wq_g[:], wqt_v[:, :, ob:ob + OG])
                wk_g = wgp.tile([128, KT, OG], F32R, tag="w")
                nc.gpsimd.dma_start(wk_g[:], wkt_v[:, :, ob:ob + OG])
                wv_g = wgp.tile([128, KT, OG], F32R, tag="w")
                nc.gpsimd.dma_start(wv_g[:], wvt_v[:, :, ob:ob + OG])

                # ---- q projection for this group: qT_g[o_local, m] ----
                qT_g = qtgp.tile([128, 2, M_SH], F32R, tag="qtg")
                for ot in range(2):
                    ps = psp.tile([128, M_SH], F32, tag="mm")
                    for kt in range(KT):
                        nc.tensor.matmul(
                            ps[:], wq_g[:, kt, ot * 128:(ot + 1) * 128],
                            qt_sb[:, kt], start=(kt == 0), stop=(kt == KT - 1))
                    nc.vector.tensor_scalar_add(
                        qT_g[:, ot], ps[:], bq_sb[:, 2 * g + ot, None])

                # ---- fused k/v projections, streaming KVt by 512-col chunk --
                kT_g = ktgp.tile([128, 2, N], F32R, tag="ktg")
                v_big = vbigp.tile([128, N // 128, HPG, DH + 1], F32R, tag="v")
                nc.gpsimd.dma_start(
                    v_big[:, :, :, DH],
                    onesc.rearrange("(a b) -> a b", a=N // 128)[None]
                    .to_broadcast((128, N // 128, HPG)))
                for ch in range(N // 512):
                    ck = stream.tile([128, KT, 512], F32R, tag="ck")
                    nc.gpsimd.dma_start(ck[:], kvt_v[:, :, ch * 512:(ch + 1) * 512])
                    for ot in range(2):
                        ps = psp.tile([128, 512], F32, tag="mm")
                        for kt in range(KT):
                            nc.tensor.matmul(
                                ps[:], wk_g[:, kt, ot * 128:(ot + 1) * 128],
                                ck[:, kt], start=(kt == 0), stop=(kt == KT - 1))
                        nc.vector.tensor_scalar_add(
                            kT_g[:, ot, ch * 512:(ch + 1) * 512], ps[:],
                            bk_sb[:, 2 * g + ot, None])
                    for ntl in range(4):
                        nt = ch * 4 + ntl
                        psv = psp.tile([128, 512], F32, tag="mm")
                        for kt in range(KT):
                            nc.tensor.matmul(
                                psv[:, 0:OG],
                                ck[:, kt, ntl * 128:(ntl + 1) * 128],
                                wv_g[:, kt], start=(kt == 0), stop=(kt == KT - 1))
                        nc.vector.tensor_add(
                            out=v_big[:, nt, :, 0:DH],
                            in0=psv[:, 0:OG].rearrange("p (j d) -> p j d", j=HPG),
                            in1=bv_b[:, ob:ob + OG].rearrange(
                                "p (j d) -> p j d", j=HPG))

                # ---- attention for the group's 4 heads ----
                pv_ps = [pvp.tile([DH + 1, M_SH], F32, tag="pv",
                                  name=f"pv_{g}_{j}") for j in range(HPG)]
                for nt in range(N // 128):
                    for j in range(HPG):
                        base, tl = (j % 2) * 64, j // 2
                        ps_s = psp.tile([128, M_SH], F32, tag="mm")
                        nc.tensor.matmul(
                            ps_s[:],
                            kT_g[base:base + 64, tl, nt * 128:(nt + 1) * 128],
                            qT_g[base:base + 64, tl, :],
                            start=True, stop=True)
                        pt_t = ptp.tile([128, M_SH], F32R, tag="pt")
                        nc.scalar.activation(
                            out=pt_t[:], in_=ps_s[:], func=AFT.Exp,
                            bias=gate_sb[:, nt, None], scale=SCALE)
                        nc.tensor.matmul(
                            pv_ps[j][:], v_big[:, nt, j, :], pt_t[:],
                            start=(nt == 0), stop=(nt == N // 128 - 1))
                # normalise by the accumulated denominator row and pack into aot
                for j in range(HPG):
                    recip = small.tile([1, M_SH], F32R, tag="recip")
                    with nc.allow_low_precision(
                            reason="fp32r operand for PE broadcast matmul"):
                        nc.vector.reciprocal(recip[:], pv_ps[j][DH:DH + 1, :])
                    ps_b = psp.tile([128, M_SH], F32, tag="mm")
                    nc.tensor.matmul(ps_b[0:DH, :], ones1[:], recip[:],
                                     start=True, stop=True)
                    rb = small.tile([DH, M_SH], F32, tag="rb")
                    nc.scalar.activation(out=rb[:], in_=ps_b[0:DH, :],
                                         func=AFT.Copy, bias=0.0, scale=1.0)
                    ao_t = small.tile([DH, M_SH], F32R, tag="aot_tmp")
                    nc.vector.tensor_mul(out=ao_t[:], in0=pv_ps[j][0:DH, :],
                                         in1=rb[:])
                    pb = (j % 2) * 64
                    nc.sync.dma_start(
                        aot[pb:pb + DH, 2 * g + j // 2, :], ao_t[:])

            # ---- output projection + bias + residual + LayerNorm ----
            wo_c = []
            for oc in range(2):
                w = stream.tile([128, KT, 512], F32R, tag="ck")
                nc.gpsimd.dma_start(w[:], wot_v[:, :, oc * 512:(oc + 1) * 512])
                wo_c.append(w)
            for mt in range(M_SH // 128):
                x_t = outst.tile([128, D], F32, tag="x")
                qn_t = outst.tile([128, D], F32, tag="qn")
                nc.sync.dma_start(qn_t[:], qn[mt * 128:(mt + 1) * 128, :])
                for oc in range(2):
                    ps = psp.tile([128, 512], F32, tag="mm")
                    for kt in range(KT):
                        nc.tensor.matmul(
                            ps[:], aot[:, kt, mt * 128:(mt + 1) * 128],
                            wo_c[oc][:, kt], start=(kt == 0), stop=(kt == KT - 1))
                    nc.vector.tensor_add(out=x_t[:, oc * 512:(oc + 1) * 512],
                                         in0=ps[:],
                                         in1=bo_b[:, oc * 512:(oc + 1) * 512])
                nc.vector.tensor_add(out=x_t[:], in0=x_t[:], in1=qn_t[:])
                # LayerNorm over D=1024 (two bn_stats subgroups of 512)
                st = outst.tile([128, 2, 6], F32, tag="st")
                nc.vector.bn_stats(st[:, 0], x_t[:, 0:512])
                nc.vector.bn_stats(st[:, 1], x_t[:, 512:1024])
                mv = outst.tile([128, 2], F32, tag="mv")
                nc.vector.bn_aggr(mv[:], st[:])
                nm = outst.tile([128, 1], F32, tag="nm")
                nc.vector.tensor_scalar_mul(nm[:], mv[:, 0:1], -1.0)
                rstd = outst.tile([128, 1], F32, tag="rstd")
                nc.scalar.activation(out=rstd[:], in_=mv[:, 1:2],
                                     func=AFT.Sqrt, bias=eps_sb[:], scale=1.0)
                nc.vector.reciprocal(rstd[:], rstd[:])
                nc.vector.tensor_scalar_add(x_t[:], x_t[:], nm[:])
                nc.vector.tensor_scalar_mul(x_t[:], x_t[:], rstd[:])
                nc.vector.tensor_mul(out=x_t[:], in0=x_t[:], in1=gamma_b[:])
                nc.vector.tensor_add(out=x_t[:], in0=x_t[:], in1=beta_b[:])
                nc.sync.dma_start(out[mt * 128:(mt + 1) * 128, :], x_t[:])

    nc.compile()
    return nc


def make_in_maps(inputs):
    f = lambda x: np.ascontiguousarray(np.asarray(x, dtype=np.float32))
    Q, KV = f(inputs["Q"]), f(inputs["KV"])
    gate = f(inputs["log_gate_bias"])
    wqt = f(np.asarray(inputs["Wq"]).T)
    wkt = f(np.asarray(inputs["Wk"]).T)
    wvt = f(np.asarray(inputs["Wv"]).T)
    wot = f(np.asarray(inputs["Wo"]).T)
    shared = {
        "wqt": wqt, "wkt": wkt, "wvt": wvt, "wot": wot,
        "bq": f(inputs["bq"]), "bk": f(inputs["bk"]),
        "bv": f(inputs["bv"]), "bo": f(inputs["bo"]),
        "gamma": f(inputs["gamma"]), "beta": f(inputs["beta"]),
        "onesc": np.ones(DH, dtype=np.float32),
    }
    in_maps = []
    for c in range(N_CORES):
        b, mh = c // 2, c % 2
        qt_b = np.ascontiguousarray(Q[b].T[:, mh * M_SH:(mh + 1) * M_SH])
        in_maps.append({
            "qt": qt_b,
            "qn": np.ascontiguousarray(Q[b, mh * M_SH:(mh + 1) * M_SH, :]),
            "kvt": np.ascontiguousarray(KV[b].T),
            "gate": np.ascontiguousarray(gate[b]),
            **shared,
        })
    return in_maps


def assemble(results):
    out = np.empty((B, M, D), dtype=np.float32)
    for c in range(N_CORES):
        b, mh = c // 2, c % 2
        out[b, mh * M_SH:(mh + 1) * M_SH, :] = results[c]["out"]
    return out


def kernel(**inputs) -> np.ndarray:
    if "nc" not in _CACHE:
        _CACHE["nc"] = build_nc()
    nc = _CACHE["nc"]
    in_maps = make_in_maps(inputs)
    res = run_bass_kernel_spmd(nc, in_maps, core_ids=list(range(N_CORES)))
    return assemble(res.results)
